# revision 13
# baseline (speedup 1.0000x reference)
"""Trainium2 Bass kernel for the DTFA (dual-attention SE + threshold
decomposition) module.

Math (per batch b):
  zt = SE(mean_T(x))            # [C, F]
  zf = SE(mean_F(x))            # [C, T]
  out1[t,f] = sum_c wf[c]*zf[c,t]*zt[c,f] + bf          (rank-C matmul)
  dcomp[k]  = where(out1 > thr_k, out1, 0), k=1..23
  out[c]    = (sum_k wf2[c,k]*dcomp[k] + bf2[c]) * x[c]

Sharding: pure data-parallel, 2 batches per core on 8 cores.

Pass 1 streams the input as [128t, 8c*256f] tiles (1 MB DMAs): T-sums via
PE ones-matmuls into [1, 2048] PSUM rows, F-sums via DVE tensor_reduce.
Pass 2 processes 1024-pixel block pairs (blocks 2i, 2i+1): a PE
broadcast-matmul replicates out1 into 2x(ones + 23 thresholds) x 2
batches rows ([112, 512] PSUM, bias folded via the ones row), one fused
DVE scalar_tensor_tensor forms (x > thr) * x, a block-diagonal [bf2|wf2]
matmul produces out2 for both batches at once, and a single DVE
tensor_tensor multiplies by the re-streamed input.
"""

import numpy as np

B, C, OC, T, F = 16, 64, 16, 256, 256
N_THR = 23
N_CORES = 8
BL = B // N_CORES  # local batches per core = 2
PIX = T * F        # 65536 per batch
NPAIR = 64         # pairs of adjacent 512-pix blocks (t-quads) per batch

_CACHE = {}


def _host_constants(w1, b1, w2, b2, wf, bf, wf2, bf2):
    f32 = np.float32
    c = {}
    # SE weights. lhsT layout [K, M]; fold the 1/256 mean scale into w1.
    c["w1Ts"] = np.ascontiguousarray(w1.T.astype(f32) / 256.0)          # [64, 16]
    c["w2T"] = np.ascontiguousarray(w2.T.astype(f32))                   # [16, 64]
    c["b1c"] = b1.astype(f32).reshape(OC, 1)
    c["b2c"] = b2.astype(f32).reshape(C, 1)
    c["wfcol"] = wf.astype(f32).reshape(C, 1)                            # [64, 1]
    bf_s = float(np.asarray(bf).reshape(-1)[0])

    # Broadcast matmul weights: xB[m, n] = sum_k bcastW[k, m] * xflat[k, n]
    # xflat rows: 0 = A even-block pix, 1 = B even, 2 = A odd, 3 = B odd,
    #             4 = ones.
    # xB rows m: 0-47 -> even block (g=0), 64-111 -> odd block (g=1);
    # within a 48-group: r = b*24 + k.  k=0 is the bias/ones row.
    bcastW = np.zeros((5, 112), f32)
    for m in range(112):
        if 48 <= m < 64:
            continue
        g, r = (0, m) if m < 48 else (1, m - 64)
        b_loc, k = divmod(r, 24)
        if k == 0:
            bcastW[4, m] = 1.0
        else:
            bcastW[2 * g + b_loc, m] = 1.0
            bcastW[4, m] = bf_s
    c["bcastW"] = bcastW

    # Threshold column for the fused (x > thr) * x op.
    thrcol = np.zeros((112, 1), f32)
    for m in range(112):
        if 48 <= m < 64:
            thrcol[m, 0] = 1e30
            continue
        r = m if m < 48 else m - 64
        k = r % 24
        thrcol[m, 0] = -1e30 if k == 0 else f32(k * (k + 1) / 600.0)
    c["thrcol"] = thrcol

    # Block-diagonal [bf2 | wf2] weights for the decomposition matmul.
    # rows (b, k) at bases 0 and 64; cols m = b*64 + c.
    wbd = np.zeros((112, 128), f32)
    for base in (0, 64):
        for b_loc in range(2):
            for k in range(24):
                row = base + 24 * b_loc + k
                cols = slice(64 * b_loc, 64 * b_loc + 64)
                wbd[row, cols] = bf2.astype(f32) if k == 0 else wf2[:, k - 1].astype(f32)
    c["wbd"] = wbd

    c["ones128"] = np.ones((128, 1), f32)
    c["ident128"] = np.eye(128, dtype=f32)
    c["ones8k"] = np.ones((1, 8192), f32)
    return c


CONST_SHAPES = {
    "w1Ts": (64, 16), "w2T": (16, 64), "b1c": (OC, 1), "b2c": (C, 1),
    "wfcol": (C, 1), "bcastW": (5, 112), "thrcol": (112, 1),
    "wbd": (112, 128), "ones128": (128, 1), "ident128": (128, 128),
    "ones8k": (1, 8192),
}


def _build_nc():
    from contextlib import ExitStack

    import concourse.bass as bass
    import concourse.bacc as bacc
    import concourse.tile as tile
    from concourse import mybir

    f32 = mybir.dt.float32
    Alu = mybir.AluOpType
    Act = mybir.ActivationFunctionType

    nc = bacc.Bacc("TRN2", target_bir_lowering=False, debug=False)
    feat = nc.dram_tensor("feat", [BL, C, T, F], f32, kind="ExternalInput")
    outp = nc.dram_tensor("outp", [BL, C, T, F], f32, kind="ExternalOutput")
    cts = {
        name: nc.dram_tensor(name, list(shape), f32, kind="ExternalInput")
        for name, shape in CONST_SHAPES.items()
    }

    with tile.TileContext(nc) as tc, ExitStack() as ctx:
        cpool = ctx.enter_context(tc.tile_pool(name="consts", bufs=1))
        sb = {}
        for name, shape in CONST_SHAPES.items():
            t = cpool.tile(list(shape), f32, tag=name, name=f"c_{name}")
            nc.gpsimd.dma_start(out=t[:], in_=cts[name][:])
            sb[name] = t

        persist = ctx.enter_context(tc.tile_pool(name="persist", bufs=1))
        p1pool = ctx.enter_context(tc.tile_pool(name="p1feat", bufs=4))

        ztsum = [persist.tile([C, F], f32, tag=f"ztsum{b}", name=f"ztsum{b}")
                 for b in range(BL)]
        zfpart = [
            [persist.tile([128, C], f32, tag=f"zfpart{b}{h}", name=f"zfpart{b}{h}")
             for h in range(2)]
            for b in range(BL)
        ]
        zfsum = [persist.tile([C, T], f32, tag=f"zfsum{b}", name=f"zfsum{b}")
                 for b in range(BL)]
        x_sb = [
            [persist.tile([128, F], f32, tag=f"xsb{b}{m}", name=f"xsb{b}{m}")
             for m in range(2)]
            for b in range(BL)
        ]

        # ---------------- Pass 1: row/col sums ----------------
        # T-sums accumulate in [1, 2048] PSUM rows (8 channels each), then
        # ACT-copy to an SBUF row and DMA-redistribute to [8, 256].
        with tc.tile_pool(name="ps_tsum", bufs=2, space="PSUM") as ppt:
            for b in range(BL):
                for jj in range(C // 8):  # octets of channels
                    tsum = ppt.tile([1, 2048], f32, tag="tsum", name="tsum")
                    for h in range(2):
                        ft = p1pool.tile([128, 8, F], f32, tag="ft", name="ft")
                        src = feat[b, 8 * jj : 8 * jj + 8, 128 * h : 128 * h + 128, :]
                        nc.sync.dma_start(out=ft[:], in_=src.transpose([1, 0, 2]))
                        for s in range(4):
                            nc.tensor.matmul(
                                tsum[:, 512 * s : 512 * s + 512],
                                sb["ones128"][:],
                                ft[:, 2 * s : 2 * s + 2, :],
                                start=(h == 0), stop=(h == 1),
                            )
                        nc.vector.tensor_reduce(
                            out=zfpart[b][h][:, 8 * jj : 8 * jj + 8],
                            in_=ft[:],
                            axis=mybir.AxisListType.X,
                            op=Alu.add,
                        )
                    ztrow = persist.tile([1, 2048], f32, tag="ztrow",
                                         name="ztrow", bufs=2)
                    nc.scalar.copy(ztrow[:], tsum[:])
                    nc.gpsimd.dma_start(
                        out=ztsum[b][8 * jj : 8 * jj + 8, :], in_=ztrow[:]
                    )

        with tc.tile_pool(name="ps_tp", bufs=2, space="PSUM") as pptp:
            for b in range(BL):
                for h in range(2):
                    tp = pptp.tile([C, 128], f32, tag="tp")
                    nc.tensor.transpose(tp[:], zfpart[b][h][:], sb["ident128"][:])
                    nc.scalar.copy(zfsum[b][:, 128 * h : 128 * h + 128], tp[:])

        # ---------------- SE branches + out1 ----------------
        def se_branch(zin, tag):
            h1p = ppse.tile([OC, 256], f32, tag="h1p")
            nc.tensor.matmul(h1p[:], sb["w1Ts"][:], zin[:])
            h1s = persist.tile([OC, 256], f32, tag=f"h1s_{tag}")
            nc.scalar.activation(h1s[:], h1p[:], Act.Relu,
                                 bias=sb["b1c"][:], scale=1.0)
            h2p = ppse.tile([C, 256], f32, tag="h2p")
            nc.tensor.matmul(h2p[:], sb["w2T"][:], h1s[:])
            zout = persist.tile([C, 256], f32, tag=f"z_{tag}")
            nc.scalar.activation(zout[:], h2p[:], Act.Sigmoid,
                                 bias=sb["b2c"][:], scale=1.0)
            return zout

        with tc.tile_pool(name="ps_se", bufs=1, space="PSUM") as ppse:
            for b in range(BL):
                zt = se_branch(ztsum[b], f"zt{b}")
                zf = se_branch(zfsum[b], f"zf{b}")
                wfzf = persist.tile([C, T], f32, tag=f"wfzf{b}")
                nc.vector.tensor_scalar_mul(wfzf[:], zf[:], sb["wfcol"][:])
                for m in range(2):
                    o1 = ppse.tile([128, F], f32, tag="o1")
                    nc.tensor.matmul(
                        o1[:], wfzf[:, 128 * m : 128 * m + 128], zt[:]
                    )
                    nc.scalar.copy(x_sb[b][m][:], o1[:])

        # ---------------- x_flat: [5, 8192] per quarter ----------------
        # Quarter q covers pairs 16q..16q+15 (t-rows 64q..64q+63).  Row
        # layout: 0 = A even blocks, 1 = B even, 2 = A odd, 3 = B odd,
        # 4 = ones.  Even block of pair p = t-rows {4p, 4p+1}; odd =
        # {4p+2, 4p+3}.
        xfpool = ctx.enter_context(tc.tile_pool(name="xflat", bufs=2))
        xflat = []
        for q in range(4):
            xf = xfpool.tile([5, 8192], f32, tag="xf", name=f"xf{q}")
            m, tbase = divmod(q, 2)  # x_sb half-tile and row base (0/64)
            for par, (b_loc, off) in enumerate(
                [(0, 0), (1, 0), (0, 2), (1, 2)]
            ):
                srct = x_sb[b_loc][m]
                pitch = srct[:].ap[0][0]
                for sub in range(2):
                    row0 = 64 * tbase + off + sub
                    s0 = srct[row0 : row0 + 1, :]
                    src_ap = bass.AP(
                        tensor=s0.tensor, offset=s0.offset,
                        ap=[[4 * pitch, 16], [1, 256]],
                    )
                    d0 = xf[par : par + 1, :]
                    dst_ap = bass.AP(
                        tensor=d0.tensor, offset=d0.offset + 256 * sub,
                        ap=[[8192, 1], [512, 16], [1, 256]],
                    )
                    nc.gpsimd.dma_start(out=dst_ap, in_=src_ap)
            nc.gpsimd.dma_start(out=xf[4:5, :], in_=cts["ones8k"][:])
            xflat.append(xf)

        # ---------------- Pass 2 ----------------
        p2pool = ctx.enter_context(tc.tile_pool(name="p2feat", bufs=4))
        opool = ctx.enter_context(tc.tile_pool(name="outs", bufs=4))
        xbspool = ctx.enter_context(tc.tile_pool(name="xbs", bufs=2))
        dcpool = ctx.enter_context(tc.tile_pool(name="dcomp", bufs=2))
        ppxb = ctx.enter_context(tc.tile_pool(name="ps_xb", bufs=2, space="PSUM"))
        ppg = ctx.enter_context(tc.tile_pool(name="ps_g", bufs=2, space="PSUM"))

        for i in range(NPAIR):
            q, r = divmod(i, 16)
            xB = ppxb.tile([112, 512], f32, tag="xB")
            nc.tensor.matmul(
                xB[:], sb["bcastW"][:], xflat[q][:, 512 * r : 512 * r + 512]
            )
            xBs = xbspool.tile([112, 512], f32, tag="xBs")
            nc.scalar.copy(xBs[:], xB[:])
            dc = dcpool.tile([112, 512], f32, tag="dc")
            nc.vector.scalar_tensor_tensor(
                out=dc[:], in0=xBs[:], scalar=sb["thrcol"][:], in1=xB[:],
                op0=Alu.is_gt, op1=Alu.mult,
            )
            gp = ppg.tile([128, 1024], f32, tag="gp")
            for g in (0, 1):
                nc.tensor.matmul(
                    gp[:, 512 * g : 512 * g + 512],
                    sb["wbd"][64 * g : 64 * g + 48, :],
                    dc[64 * g : 64 * g + 48, :],
                )
            ft2 = p2pool.tile([128, 4, F], f32, tag="ft2")
            nc.sync.dma_start(out=ft2[:], in_=feat[:, :, 4 * i : 4 * i + 4, :])
            ot = opool.tile([128, 4, F], f32, tag="ot")
            nc.vector.tensor_tensor(
                out=ot[:], in0=gp[:].rearrange("p (a b) -> p a b", a=4),
                in1=ft2[:], op=Alu.mult,
            )
            nc.scalar.dma_start(out=outp[:, :, 4 * i : 4 * i + 4, :], in_=ot[:])

    nc.finalize()
    return nc


def _get_nc():
    if "nc" not in _CACHE:
        _CACHE["nc"] = _build_nc()
    return _CACHE["nc"]


def _make_runner(nc, n_cores):
    """Cached jitted shard_map executor for `nc` (mirrors
    bass2jax.run_bass_via_pjrt but reusable across calls)."""
    import jax
    from jax.sharding import Mesh, PartitionSpec
    from jax.experimental.shard_map import shard_map
    from concourse import bass2jax, mybir

    bass2jax.install_neuronx_cc_hook()

    partition_name = (
        nc.partition_id_tensor.name if nc.partition_id_tensor else None
    )
    in_names, out_names, out_avals, zero_outs = [], [], [], []
    for alloc in nc.m.functions[0].allocations:
        if not isinstance(alloc, mybir.MemoryLocationSet):
            continue
        name = alloc.memorylocations[0].name
        if alloc.kind == "ExternalInput":
            if name != partition_name:
                in_names.append(name)
        elif alloc.kind == "ExternalOutput":
            out_names.append(name)
            shape = tuple(alloc.tensor_shape)
            dtype = mybir.dt.np(alloc.dtype)
            out_avals.append(jax.core.ShapedArray(shape, dtype))
            zero_outs.append(np.zeros(shape, dtype))
    n_params = len(in_names)
    all_in_names = in_names + out_names
    if partition_name is not None:
        all_in_names = all_in_names + [partition_name]
    donate = tuple(range(n_params, n_params + len(out_names)))

    def _body(*args):
        operands = list(args)
        if partition_name is not None:
            operands.append(bass2jax.partition_id_tensor())
        outs = bass2jax._bass_exec_p.bind(
            *operands,
            out_avals=tuple(out_avals),
            in_names=tuple(all_in_names),
            out_names=tuple(out_names),
            lowering_input_output_aliases=(),
            sim_require_finite=True,
            sim_require_nnan=True,
            nc=nc,
        )
        return tuple(outs)

    devices = jax.devices()[:n_cores]
    mesh = Mesh(np.asarray(devices), ("core",))
    specs = (PartitionSpec("core"),) * (n_params + len(out_names))
    sharded = jax.jit(
        shard_map(_body, mesh=mesh, in_specs=specs,
                  out_specs=(PartitionSpec("core"),) * len(out_names),
                  check_rep=False),
        donate_argnums=donate, keep_unused=True,
    )

    def run(in_maps):
        per_core = [[np.asarray(m[name]) for name in in_names] for m in in_maps]
        concat_in = [
            np.concatenate([per_core[c][i] for c in range(n_cores)], axis=0)
            for i in range(n_params)
        ]
        concat_zeros = [
            np.zeros((n_cores * z.shape[0], *z.shape[1:]), z.dtype)
            for z in zero_outs
        ]
        out_arrs = sharded(*concat_in, *concat_zeros)
        return [
            {
                name: np.asarray(out_arrs[i]).reshape(n_cores, *out_avals[i].shape)[c]
                for i, name in enumerate(out_names)
            }
            for c in range(n_cores)
        ]

    run.sharded = sharded
    run.in_names = in_names
    run.out_names = out_names
    run.zero_outs = zero_outs
    run.n_params = n_params
    return run


def _get_runner():
    if "runner" not in _CACHE:
        _CACHE["runner"] = _make_runner(_get_nc(), N_CORES)
    return _CACHE["runner"]


def kernel(**inputs):
    feature_in = np.ascontiguousarray(np.asarray(inputs["feature_in"], np.float32))
    consts = _host_constants(
        np.asarray(inputs["w1"]), np.asarray(inputs["b1"]),
        np.asarray(inputs["w2"]), np.asarray(inputs["b2"]),
        np.asarray(inputs["wf"]), np.asarray(inputs["bf"]),
        np.asarray(inputs["wf2"]), np.asarray(inputs["bf2"]),
    )
    in_maps = []
    for core in range(N_CORES):
        m = {"feat": feature_in[BL * core : BL * core + BL]}
        m.update(consts)
        in_maps.append(m)

    run = _get_runner()
    res = run(in_maps)
    out = np.concatenate([res[c]["outp"] for c in range(N_CORES)], axis=0)
    return out.reshape(B, C, T, F).astype(np.float32)


# revision 15
# speedup vs baseline: 17443.8621x; 17443.8621x over previous
"""Trainium2 Bass kernel for the DTFA (dual-attention SE + threshold
decomposition) module.

Math (per batch b):
  zt = SE(mean_T(x))            # [C, F]
  zf = SE(mean_F(x))            # [C, T]
  out1[t,f] = sum_c wf[c]*zf[c,t]*zt[c,f] + bf          (rank-C matmul)
  dcomp[k]  = where(out1 > thr_k, out1, 0), k=1..23
  out[c]    = (sum_k wf2[c,k]*dcomp[k] + bf2[c]) * x[c]

Sharding: pure data-parallel, 2 batches per core on 8 cores.

Pass 1 streams the input as [128t, 8c*256f] tiles (1 MB DMAs): T-sums via
PE ones-matmuls into [1, 2048] PSUM rows, F-sums via DVE tensor_reduce.
Pass 2 processes 1024-pixel block pairs (blocks 2i, 2i+1): a PE
broadcast-matmul replicates out1 into 2x(ones + 23 thresholds) x 2
batches rows ([112, 512] PSUM, bias folded via the ones row), one fused
DVE scalar_tensor_tensor forms (x > thr) * x, a block-diagonal [bf2|wf2]
matmul produces out2 for both batches at once, and a single DVE
tensor_tensor multiplies by the re-streamed input.
"""

import numpy as np

B, C, OC, T, F = 16, 64, 16, 256, 256
N_THR = 23
N_CORES = 8
BL = B // N_CORES  # local batches per core = 2
PIX = T * F        # 65536 per batch
NPAIR = 64         # pairs of adjacent 512-pix blocks (t-quads) per batch

_CACHE = {}


def _host_constants(w1, b1, w2, b2, wf, bf, wf2, bf2):
    f32 = np.float32
    c = {}
    # SE weights. lhsT layout [K, M]; fold the 1/256 mean scale into w1.
    c["w1Ts"] = np.ascontiguousarray(w1.T.astype(f32) / 256.0)          # [64, 16]
    c["w2T"] = np.ascontiguousarray(w2.T.astype(f32))                   # [16, 64]
    c["b1c"] = b1.astype(f32).reshape(OC, 1)
    c["b2c"] = b2.astype(f32).reshape(C, 1)
    c["wfcol"] = wf.astype(f32).reshape(C, 1)                            # [64, 1]
    bf_s = float(np.asarray(bf).reshape(-1)[0])

    # Broadcast matmul weights: xB[m, n] = sum_k bcastW[k, m] * xflat[k, n]
    # xflat rows: 0 = A even-block pix, 1 = B even, 2 = A odd, 3 = B odd,
    #             4 = ones.
    # xB rows m: 0-47 -> even block (g=0), 64-111 -> odd block (g=1);
    # within a 48-group: r = b*24 + k.  k=0 is the bias/ones row.
    bcastW = np.zeros((5, 112), f32)
    for m in range(112):
        if 48 <= m < 64:
            continue
        g, r = (0, m) if m < 48 else (1, m - 64)
        b_loc, k = divmod(r, 24)
        if k == 0:
            bcastW[4, m] = 1.0
        else:
            bcastW[2 * g + b_loc, m] = 1.0
            bcastW[4, m] = bf_s
    c["bcastW"] = bcastW

    # Threshold column for the fused (x > thr) * x op.
    thrcol = np.zeros((112, 1), f32)
    for m in range(112):
        if 48 <= m < 64:
            thrcol[m, 0] = 1e30
            continue
        r = m if m < 48 else m - 64
        k = r % 24
        thrcol[m, 0] = -1e30 if k == 0 else f32(k * (k + 1) / 600.0)
    c["thrcol"] = thrcol

    # Block-diagonal [bf2 | wf2] weights for the decomposition matmul.
    # rows (b, k) at bases 0 and 64; cols m = b*64 + c.
    wbd = np.zeros((112, 128), f32)
    for base in (0, 64):
        for b_loc in range(2):
            for k in range(24):
                row = base + 24 * b_loc + k
                cols = slice(64 * b_loc, 64 * b_loc + 64)
                wbd[row, cols] = bf2.astype(f32) if k == 0 else wf2[:, k - 1].astype(f32)
    c["wbd"] = wbd

    c["ones128"] = np.ones((128, 1), f32)
    c["ident128"] = np.eye(128, dtype=f32)
    c["ones8k"] = np.ones((1, 8192), f32)
    return c


CONST_SHAPES = {
    "w1Ts": (64, 16), "w2T": (16, 64), "b1c": (OC, 1), "b2c": (C, 1),
    "wfcol": (C, 1), "bcastW": (5, 112), "thrcol": (112, 1),
    "wbd": (112, 128), "ones128": (128, 1), "ident128": (128, 128),
    "ones8k": (1, 8192),
}


def _build_nc(reps=1):
    from contextlib import ExitStack, nullcontext

    import concourse.bass as bass
    import concourse.bacc as bacc
    import concourse.tile as tile
    from concourse import mybir

    f32 = mybir.dt.float32
    Alu = mybir.AluOpType
    Act = mybir.ActivationFunctionType

    nc = bacc.Bacc("TRN2", target_bir_lowering=False, debug=False)
    feat = nc.dram_tensor("feat", [BL, C, T, F], f32, kind="ExternalInput")
    outp = nc.dram_tensor("outp", [BL, C, T, F], f32, kind="ExternalOutput")
    cts = {
        name: nc.dram_tensor(name, list(shape), f32, kind="ExternalInput")
        for name, shape in CONST_SHAPES.items()
    }

    with tile.TileContext(nc) as tc, ExitStack() as ctx:
        cpool = ctx.enter_context(tc.tile_pool(name="consts", bufs=1))
        sb = {}
        for name, shape in CONST_SHAPES.items():
            t = cpool.tile(list(shape), f32, tag=name, name=f"c_{name}")
            nc.gpsimd.dma_start(out=t[:], in_=cts[name][:])
            sb[name] = t

        loop_cm = tc.For_i(0, reps, 1) if reps > 1 else nullcontext()
        ctx.enter_context(loop_cm)
        persist = ctx.enter_context(tc.tile_pool(name="persist", bufs=1))
        p1pool = ctx.enter_context(tc.tile_pool(name="p1feat", bufs=4))

        ztsum = [persist.tile([C, F], f32, tag=f"ztsum{b}", name=f"ztsum{b}")
                 for b in range(BL)]
        zfpart = [
            [persist.tile([128, C], f32, tag=f"zfpart{b}{h}", name=f"zfpart{b}{h}")
             for h in range(2)]
            for b in range(BL)
        ]
        zfsum = [persist.tile([C, T], f32, tag=f"zfsum{b}", name=f"zfsum{b}")
                 for b in range(BL)]
        x_sb = [
            [persist.tile([128, F], f32, tag=f"xsb{b}{m}", name=f"xsb{b}{m}")
             for m in range(2)]
            for b in range(BL)
        ]

        # ---------------- Pass 1: row/col sums ----------------
        # T-sums accumulate in [1, 2048] PSUM rows (8 channels each), then
        # ACT-copy to an SBUF row and DMA-redistribute to [8, 256].
        with tc.tile_pool(name="ps_tsum", bufs=2, space="PSUM") as ppt:
            for b in range(BL):
                for jj in range(C // 8):  # octets of channels
                    tsum = ppt.tile([1, 2048], f32, tag="tsum", name="tsum")
                    for h in range(2):
                        ft = p1pool.tile([128, 8, F], f32, tag="ft", name="ft")
                        src = feat[b, 8 * jj : 8 * jj + 8, 128 * h : 128 * h + 128, :]
                        nc.sync.dma_start(out=ft[:], in_=src.transpose([1, 0, 2]))
                        for s in range(4):
                            nc.tensor.matmul(
                                tsum[:, 512 * s : 512 * s + 512],
                                sb["ones128"][:],
                                ft[:, 2 * s : 2 * s + 2, :],
                                start=(h == 0), stop=(h == 1),
                            )
                        nc.vector.tensor_reduce(
                            out=zfpart[b][h][:, 8 * jj : 8 * jj + 8],
                            in_=ft[:],
                            axis=mybir.AxisListType.X,
                            op=Alu.add,
                        )
                    ztrow = persist.tile([1, 2048], f32, tag="ztrow",
                                         name="ztrow", bufs=2)
                    nc.scalar.copy(ztrow[:], tsum[:])
                    nc.gpsimd.dma_start(
                        out=ztsum[b][8 * jj : 8 * jj + 8, :], in_=ztrow[:]
                    )

        with tc.tile_pool(name="ps_tp", bufs=2, space="PSUM") as pptp:
            for b in range(BL):
                for h in range(2):
                    tp = pptp.tile([C, 128], f32, tag="tp")
                    nc.tensor.transpose(tp[:], zfpart[b][h][:], sb["ident128"][:])
                    nc.scalar.copy(zfsum[b][:, 128 * h : 128 * h + 128], tp[:])

        # ---------------- SE branches + out1 ----------------
        def se_branch(zin, tag):
            h1p = ppse.tile([OC, 256], f32, tag="h1p")
            nc.tensor.matmul(h1p[:], sb["w1Ts"][:], zin[:])
            h1s = persist.tile([OC, 256], f32, tag=f"h1s_{tag}")
            nc.scalar.activation(h1s[:], h1p[:], Act.Relu,
                                 bias=sb["b1c"][:], scale=1.0)
            h2p = ppse.tile([C, 256], f32, tag="h2p")
            nc.tensor.matmul(h2p[:], sb["w2T"][:], h1s[:])
            zout = persist.tile([C, 256], f32, tag=f"z_{tag}")
            nc.scalar.activation(zout[:], h2p[:], Act.Sigmoid,
                                 bias=sb["b2c"][:], scale=1.0)
            return zout

        with tc.tile_pool(name="ps_se", bufs=1, space="PSUM") as ppse:
            for b in range(BL):
                zt = se_branch(ztsum[b], f"zt{b}")
                zf = se_branch(zfsum[b], f"zf{b}")
                wfzf = persist.tile([C, T], f32, tag=f"wfzf{b}")
                nc.vector.tensor_scalar_mul(wfzf[:], zf[:], sb["wfcol"][:])
                for m in range(2):
                    o1 = ppse.tile([128, F], f32, tag="o1")
                    nc.tensor.matmul(
                        o1[:], wfzf[:, 128 * m : 128 * m + 128], zt[:]
                    )
                    nc.scalar.copy(x_sb[b][m][:], o1[:])

        # ---------------- x_flat: [5, 8192] per quarter ----------------
        # Quarter q covers pairs 16q..16q+15 (t-rows 64q..64q+63).  Row
        # layout: 0 = A even blocks, 1 = B even, 2 = A odd, 3 = B odd,
        # 4 = ones.  Even block of pair p = t-rows {4p, 4p+1}; odd =
        # {4p+2, 4p+3}.
        xfpool = ctx.enter_context(tc.tile_pool(name="xflat", bufs=2))
        xflat = []
        for q in range(4):
            xf = xfpool.tile([5, 8192], f32, tag="xf", name=f"xf{q}")
            m, tbase = divmod(q, 2)  # x_sb half-tile and row base (0/64)
            for par, (b_loc, off) in enumerate(
                [(0, 0), (1, 0), (0, 2), (1, 2)]
            ):
                srct = x_sb[b_loc][m]
                pitch = srct[:].ap[0][0]
                for sub in range(2):
                    row0 = 64 * tbase + off + sub
                    s0 = srct[row0 : row0 + 1, :]
                    src_ap = bass.AP(
                        tensor=s0.tensor, offset=s0.offset,
                        ap=[[4 * pitch, 16], [1, 256]],
                    )
                    d0 = xf[par : par + 1, :]
                    dst_ap = bass.AP(
                        tensor=d0.tensor, offset=d0.offset + 256 * sub,
                        ap=[[8192, 1], [512, 16], [1, 256]],
                    )
                    nc.gpsimd.dma_start(out=dst_ap, in_=src_ap)
            nc.gpsimd.dma_start(out=xf[4:5, :], in_=cts["ones8k"][:])
            xflat.append(xf)

        # ---------------- Pass 2 ----------------
        p2pool = ctx.enter_context(tc.tile_pool(name="p2feat", bufs=4))
        opool = ctx.enter_context(tc.tile_pool(name="outs", bufs=4))
        xbspool = ctx.enter_context(tc.tile_pool(name="xbs", bufs=2))
        dcpool = ctx.enter_context(tc.tile_pool(name="dcomp", bufs=2))
        ppxb = ctx.enter_context(tc.tile_pool(name="ps_xb", bufs=2, space="PSUM"))
        ppg = ctx.enter_context(tc.tile_pool(name="ps_g", bufs=2, space="PSUM"))

        for i in range(NPAIR):
            q, r = divmod(i, 16)
            xB = ppxb.tile([112, 512], f32, tag="xB")
            nc.tensor.matmul(
                xB[:], sb["bcastW"][:], xflat[q][:, 512 * r : 512 * r + 512]
            )
            xBs = xbspool.tile([112, 512], f32, tag="xBs")
            nc.scalar.copy(xBs[:], xB[:])
            dc = dcpool.tile([112, 512], f32, tag="dc")
            nc.vector.scalar_tensor_tensor(
                out=dc[:], in0=xBs[:], scalar=sb["thrcol"][:], in1=xB[:],
                op0=Alu.is_gt, op1=Alu.mult,
            )
            gp = ppg.tile([128, 1024], f32, tag="gp")
            for g in (0, 1):
                nc.tensor.matmul(
                    gp[:, 512 * g : 512 * g + 512],
                    sb["wbd"][64 * g : 64 * g + 48, :],
                    dc[64 * g : 64 * g + 48, :],
                )
            ft2 = p2pool.tile([128, 4, F], f32, tag="ft2")
            nc.sync.dma_start(out=ft2[:], in_=feat[:, :, 4 * i : 4 * i + 4, :])
            ot = opool.tile([128, 4, F], f32, tag="ot")
            nc.vector.tensor_tensor(
                out=ot[:], in0=gp[:].rearrange("p (a b) -> p a b", a=4),
                in1=ft2[:], op=Alu.mult,
            )
            nc.scalar.dma_start(out=outp[:, :, 4 * i : 4 * i + 4, :], in_=ot[:])

    nc.finalize()
    return nc


def _get_nc(reps=1):
    key = ("nc", reps)
    if key not in _CACHE:
        _CACHE[key] = _build_nc(reps)
    return _CACHE[key]


def _make_runner(nc, n_cores):
    """Cached jitted shard_map executor for `nc` (mirrors
    bass2jax.run_bass_via_pjrt but reusable across calls)."""
    import jax
    from jax.sharding import Mesh, PartitionSpec
    from jax.experimental.shard_map import shard_map
    from concourse import bass2jax, mybir

    bass2jax.install_neuronx_cc_hook()

    partition_name = (
        nc.partition_id_tensor.name if nc.partition_id_tensor else None
    )
    in_names, out_names, out_avals, zero_outs = [], [], [], []
    for alloc in nc.m.functions[0].allocations:
        if not isinstance(alloc, mybir.MemoryLocationSet):
            continue
        name = alloc.memorylocations[0].name
        if alloc.kind == "ExternalInput":
            if name != partition_name:
                in_names.append(name)
        elif alloc.kind == "ExternalOutput":
            out_names.append(name)
            shape = tuple(alloc.tensor_shape)
            dtype = mybir.dt.np(alloc.dtype)
            out_avals.append(jax.core.ShapedArray(shape, dtype))
            zero_outs.append(np.zeros(shape, dtype))
    n_params = len(in_names)
    all_in_names = in_names + out_names
    if partition_name is not None:
        all_in_names = all_in_names + [partition_name]
    donate = tuple(range(n_params, n_params + len(out_names)))

    def _body(*args):
        operands = list(args)
        if partition_name is not None:
            operands.append(bass2jax.partition_id_tensor())
        outs = bass2jax._bass_exec_p.bind(
            *operands,
            out_avals=tuple(out_avals),
            in_names=tuple(all_in_names),
            out_names=tuple(out_names),
            lowering_input_output_aliases=(),
            sim_require_finite=True,
            sim_require_nnan=True,
            nc=nc,
        )
        return tuple(outs)

    devices = jax.devices()[:n_cores]
    mesh = Mesh(np.asarray(devices), ("core",))
    specs = (PartitionSpec("core"),) * (n_params + len(out_names))
    sharded = jax.jit(
        shard_map(_body, mesh=mesh, in_specs=specs,
                  out_specs=(PartitionSpec("core"),) * len(out_names),
                  check_rep=False),
        donate_argnums=donate, keep_unused=True,
    )

    def run(in_maps):
        per_core = [[np.asarray(m[name]) for name in in_names] for m in in_maps]
        concat_in = [
            np.concatenate([per_core[c][i] for c in range(n_cores)], axis=0)
            for i in range(n_params)
        ]
        concat_zeros = [
            np.zeros((n_cores * z.shape[0], *z.shape[1:]), z.dtype)
            for z in zero_outs
        ]
        out_arrs = sharded(*concat_in, *concat_zeros)
        return [
            {
                name: np.asarray(out_arrs[i]).reshape(n_cores, *out_avals[i].shape)[c]
                for i, name in enumerate(out_names)
            }
            for c in range(n_cores)
        ]

    def make_chain(n_reps):
        """Jitted callable running the kernel n_reps times back-to-back on
        device (each rep's outputs become the next rep's output buffers),
        for overhead-free timing via slope."""
        def _bodyN(*args):
            ins = list(args[:n_params])
            outs = list(args[n_params:])
            for _ in range(n_reps):
                outs = list(_body(*ins, *outs))
            return tuple(outs)

        return jax.jit(
            shard_map(_bodyN, mesh=mesh, in_specs=specs,
                      out_specs=(PartitionSpec("core"),) * len(out_names),
                      check_rep=False),
            keep_unused=True,
        )

    run.sharded = sharded
    run.in_names = in_names
    run.out_names = out_names
    run.zero_outs = zero_outs
    run.n_params = n_params
    run.make_chain = make_chain
    return run


def _get_runner(reps=1):
    key = ("runner", reps)
    if key not in _CACHE:
        _CACHE[key] = _make_runner(_get_nc(reps), N_CORES)
    return _CACHE[key]


def kernel(**inputs):
    feature_in = np.ascontiguousarray(np.asarray(inputs["feature_in"], np.float32))
    consts = _host_constants(
        np.asarray(inputs["w1"]), np.asarray(inputs["b1"]),
        np.asarray(inputs["w2"]), np.asarray(inputs["b2"]),
        np.asarray(inputs["wf"]), np.asarray(inputs["bf"]),
        np.asarray(inputs["wf2"]), np.asarray(inputs["bf2"]),
    )
    in_maps = []
    for core in range(N_CORES):
        m = {"feat": feature_in[BL * core : BL * core + BL]}
        m.update(consts)
        in_maps.append(m)

    run = _get_runner()
    res = run(in_maps)
    out = np.concatenate([res[c]["outp"] for c in range(N_CORES)], axis=0)
    return out.reshape(B, C, T, F).astype(np.float32)
